# revision 8
# baseline (speedup 1.0000x reference)
"""ChannelAttention Trainium2 kernel.

Per-sample computation (B=8, one sample per NeuronCore):
    K   = x[b] viewed as (C=64, N=110592)   (raw row-major reshape)
    G   = K @ K^T                            (64, 64)
    m3  = G @ G
    A   = sigmoid(m3)                        (fully saturated 0/1 in practice)
    out = gamma * (A @ K) + x[b]

Kernel structure per core (memory-bound; HBM traffic = 2 reads + 1 write):
  Phase 1 (Gram): load K via a strided DMA pattern that puts the
    contraction dim n on SBUF partitions: tile[p, c, j] = K[c, p*864 + j0 + j].
    Each (128-partition, 64-free) slice tile[:, :, j] is directly a
    K^T chunk -> 864 accumulating PE matmuls into one PSUM tile.
    The DMA descriptor runs are 512B/384B contiguous and 64B-aligned,
    so this "free transpose" streams at near line-rate.
  Middle: m3 = G@G (duplicated into both PSUM partition halves),
    sigmoid on ACT, scale by gamma (folded here so the phase-2 epilogue
    is a single DVE add).
  Phase 2: load K naturally as (128, 2048) tiles (two 2048-col chunks
    stacked on partition halves), matmul per half against the gamma-scaled
    affinity (concurrent PE quadrants (0,0)/(64,64)), epilogue
    out = W_psum + x_tile in one DVE tensor_tensor add, store.
"""

from contextlib import ExitStack

import numpy as np

import concourse.bass as bass
import concourse.mybir as mybir
import concourse.tile as tile
from concourse.bass_utils import run_bass_kernel_spmd


def split_waits(nc, max_waits: int = 1) -> int:
    """Walrus workaround: cayman instruction structs have a single
    NEURON_ISA_TPB_EVENTS slot and this walrus build rejects BIR
    instructions carrying more sync waits ("Too many sync wait commands").
    Move excess waits onto no-fuse NoOps inserted immediately before the
    instruction on the same engine — per-engine program order preserves the
    blocking semantics. Updates are never moved."""
    n_inserted = 0
    for f in nc.m.functions:
        for b in f.blocks:
            out = []
            changed = False
            for inst in b.instructions:
                si = inst.sync_info
                waits = list(si.on_wait) if si and si.on_wait else []
                if len(waits) > max_waits:
                    changed = True
                    si.on_wait = waits[:max_waits]
                    rest = waits[max_waits:]
                    for k in range(0, len(rest), max_waits):
                        n_inserted += 1
                        nop = mybir.InstNoOp(
                            name=f"I-waitsplit-{n_inserted}",
                            engine=inst.engine,
                            ins=[],
                            outs=[],
                            bass_nofuse=True,
                            sync_info=mybir.SyncInfo(
                                on_wait=rest[k : k + max_waits], on_update=[]
                            ),
                        )
                        nc.register_instruction(nop, overwrite=True)
                        out.append(nop)
                out.append(inst)
            if changed:
                b.instructions = out
    return n_inserted

F32 = mybir.dt.float32

B = 8
C = 64
N = 48 * 48 * 48  # 110592
P = 128  # SBUF partitions
L = N // P  # 864 columns of K per partition in the gram layout

# Gram-phase n_in chunking: 864 = 6*128 + 96. Runs are 512B/384B, 64B aligned.
GRAM_CHUNKS = [128] * 6 + [96]
assert sum(GRAM_CHUNKS) == L

NT = 2048  # natural-tile columns per stacked half; tile = (128, NT) = 1 MiB
N_PAIR = N // (2 * NT)  # 27
assert N_PAIR * 2 * NT == N

MM_N = 512  # fp32 moving-operand max free dim


def build_channel_attention(n: int = N) -> bass.Bass:
    l = n // P
    n_pair = n // (2 * NT) if n >= 2 * NT else 0
    assert l * P == n

    nc = bass.Bass()
    x_d = nc.dram_tensor("x", [C, n], F32, kind="ExternalInput")
    g_d = nc.dram_tensor("gamma", [1, 1], F32, kind="ExternalInput")
    o_d = nc.dram_tensor("out", [C, n], F32, kind="ExternalOutput")

    # gram chunking for this n
    chunks = []
    rem = l
    while rem > 0:
        w = min(128, rem)
        chunks.append(w)
        rem -= w

    with tile.TileContext(nc) as tc, ExitStack() as ctx:
        singles = ctx.enter_context(tc.tile_pool(name="singles", bufs=1))
        funky = ctx.enter_context(tc.tile_pool(name="funky", bufs=2))
        xin = ctx.enter_context(tc.tile_pool(name="xin", bufs=4))
        oout = ctx.enter_context(tc.tile_pool(name="oout", bufs=3))
        gram_ps = ctx.enter_context(
            tc.tile_pool(name="gram_ps", bufs=1, space="PSUM")
        )
        w_ps_pool = ctx.enter_context(tc.tile_pool(name="w_ps", bufs=3, space="PSUM"))

        # gamma broadcast to all 128 partitions (per-partition scalar operand)
        gamma_sb = singles.tile([P, 1], F32)
        nc.gpsimd.dma_start(
            out=gamma_sb,
            in_=bass.AP(tensor=g_d, offset=0, ap=[[0, P], [1, 1]]),
        )

        # ---------------- Phase 1: G = K @ K^T ----------------
        # (128, 64, l) view: xf[p, c, j] = K[c, p*l + j]
        xf = x_d[:, :].rearrange("c (p j) -> p c j", p=P)
        gram = gram_ps.tile([C, C], F32)

        n_mm = len(chunks)
        j0 = 0
        mm_i = 0
        for w in chunks:
            fk = funky.tile([P, C, 128], F32, tag="funky")
            nc.sync.dma_start(out=fk[:, :, :w], in_=xf[:, :, j0 : j0 + w])
            for j in range(w):
                nc.tensor.matmul(
                    gram,
                    fk[:, :, j],
                    fk[:, :, j],
                    start=(mm_i == 0),
                    stop=(mm_i == l - 1),
                )
                mm_i += 1
            j0 += w
        assert mm_i == l

        # ---------------- Middle: A = gamma * sigmoid(G @ G) ----------------
        g_sb = singles.tile([C, C], F32)
        nc.vector.tensor_copy(g_sb, gram)
        m3 = gram_ps.tile([P, C], F32)
        # duplicate m3 into both partition halves so phase 2 can use
        # both PE row groups (stacked rhs tiles)
        nc.tensor.matmul(m3[0:C, :], g_sb, g_sb, start=True, stop=True)
        nc.tensor.matmul(m3[C:P, :], g_sb, g_sb, start=True, stop=True)
        a2 = singles.tile([P, C], F32)
        zero_bias = singles.tile([P, 1], F32)
        nc.vector.memset(zero_bias, 0.0)
        nc.scalar.activation(
            a2, m3, mybir.ActivationFunctionType.Sigmoid, bias=zero_bias
        )
        nc.vector.tensor_scalar_mul(a2, a2, gamma_sb)

        # ---------------- Phase 2: out = (gamma*A) @ K + x ----------------
        xn = x_d[:, :].rearrange("c (t f) -> t c f", f=NT)
        on = o_d[:, :].rearrange("c (t f) -> t c f", f=NT)
        for t in range(n_pair):
            xt = xin.tile([P, NT], F32)
            nc.sync.dma_start(out=xt, in_=xn[2 * t : 2 * t + 2])
            ot = oout.tile([P, NT], F32)
            for f0 in range(0, NT, MM_N):
                w_ps = w_ps_pool.tile([P, MM_N], F32)
                nc.tensor.matmul(
                    w_ps[0:C, :],
                    a2[0:C, :],
                    xt[0:C, f0 : f0 + MM_N],
                    start=True,
                    stop=True,
                )
                nc.tensor.matmul(
                    w_ps[C:P, :],
                    a2[C:P, :],
                    xt[C:P, f0 : f0 + MM_N],
                    start=True,
                    stop=True,
                )
                nc.vector.tensor_add(
                    ot[:, f0 : f0 + MM_N], w_ps, xt[:, f0 : f0 + MM_N]
                )
            # store via the ACT HWDGE ring so stores don't queue behind
            # later loads on the SP ring
            nc.scalar.dma_start(out=on[2 * t : 2 * t + 2], in_=ot)

    split_waits(nc)
    return nc


_NC_CACHE: dict[int, bass.Bass] = {}


def _get_nc(n: int = N) -> bass.Bass:
    if n not in _NC_CACHE:
        _NC_CACHE[n] = build_channel_attention(n)
    return _NC_CACHE[n]


def kernel(x: np.ndarray, gamma: np.ndarray, **run_kwargs):
    x = np.ascontiguousarray(np.asarray(x, dtype=np.float32))
    b = x.shape[0]
    n = int(np.prod(x.shape[1:])) // C
    g11 = np.asarray(gamma, dtype=np.float32).reshape(1, 1)
    xs = x.reshape(b, C, n)

    nc = _get_nc(n)
    in_maps = [{"x": xs[i], "gamma": g11} for i in range(b)]
    res = run_bass_kernel_spmd(nc, in_maps, core_ids=list(range(b)), **run_kwargs)
    out = np.stack([res.results[i]["out"] for i in range(b)])
    if run_kwargs.get("trace"):
        kernel.last_result = res
    return out.reshape(x.shape).astype(np.float32)
